# revision 3
# baseline (speedup 1.0000x reference)
"""Trainium2 Bass kernel for GroupedQuerySelfAttention (v3, head-sharded,
software-pipelined).

Sharding: 8 cores = 2 batches x 4 kv-head pairs. Core c: batch b=c//4,
kv-heads {2t, 2t+1} (t=c%4) + their 8 query heads, full sequence. Each
core emits a partial output projection; the host sums the 4 partials
per batch and adds the bias.

Schedule (single PE stream, emission order = per-engine execution
order):
  head:  per 512-row group: load x rows, bf16-convert (gpsimd),
         PE-transpose into xbT, V proj per row-block, K proj per group.
  steady: per 512-query chunk (qch): Q proj (chunk), then 8 (g,dh)
         pairs: QK into 2-bank PSUM batches -> one Exp per batch (ACT
         is the near-critical engine; 2 qk buffers keep it saturated);
         transposed PV ([q,65] output, ones column gives per-partition
         denominators); PV chains + out-proj chunks of the previous
         pair/chunk are interleaved between QK batches so PE never
         parks on a not-yet-free qk buffer.
"""

import numpy as np
from collections import deque
from contextlib import ExitStack

import concourse.bass as bass
import concourse.tile as tile
from concourse import bacc, mybir
from concourse.bass_utils import run_bass_kernel_spmd
from concourse.masks import make_identity

P = 128
B, N, D = 2, 2048, 2048
H, G, C = 8, 4, 64
NB = N // P                   # 16 seq blocks
DB = D // P                   # 16 d blocks
NJ = 2 * G * C                # 512 output cols per core
SCALE = float(1.0 / np.sqrt(H * G))
F32 = mybir.dt.float32
BF16 = mybir.dt.bfloat16
AF = mybir.ActivationFunctionType


def build_program(n_cores=8, phases=None):
    nc = bacc.Bacc("TRN2", target_bir_lowering=False, debug=False,
                   num_devices=n_cores)
    xb = nc.dram_tensor("xb", [N, D], F32, kind="ExternalInput").ap()
    wqs = nc.dram_tensor("wqs", [D, NJ], BF16, kind="ExternalInput").ap()
    wkvs = nc.dram_tensor("wkvs", [D, 2 * P], BF16, kind="ExternalInput").ap()
    wps = nc.dram_tensor("wps", [NJ, D], BF16, kind="ExternalInput").ap()
    out = nc.dram_tensor("out", [N, D], F32, kind="ExternalOutput").ap()

    with tile.TileContext(nc) as tc, ExitStack() as top:
        # ---- persistent stores ----
        store = top.enter_context(tc.tile_pool(name="store", bufs=1))
        KT = store.tile([P, N], BF16, tag="KT")           # 4KB/part
        Vst = store.tile([P, NB, 2, C + 1], BF16, tag="Vst")
        xbT = store.tile([P, DB, N], BF16, tag="xbT")     # 64KB/part
        wps_t = store.tile([P, G, N], BF16, tag="wps")    # 16KB/part
        identb = store.tile([P, P], BF16, tag="identb")
        make_identity(nc, identb[:])
        onesb = store.tile([P, 1], BF16, tag="onesb")
        nc.gpsimd.memset(onesb[:], 1.0)
        nc.vector.tensor_copy(
            Vst[:, :, :, C:C + 1],
            onesb[:, None, None, :].to_broadcast((P, NB, 2, 1)))

        # ---- PSUM pools (8 banks total) ----
        qpp = top.enter_context(           # Q/K/V/D/PV accumulation chains
            tc.tile_pool(name="qpp", bufs=2, space="PSUM"))   # 2 banks
        qk_p = top.enter_context(
            tc.tile_pool(name="qkp", bufs=2, space="PSUM"))   # 4 banks
        tpp = top.enter_context(           # x transposes + OT transposes
            tc.tile_pool(name="tpp", bufs=2, space="PSUM"))   # 2 banks

        # ---- steady-state SBUF pools ----
        qt_p = top.enter_context(tc.tile_pool(name="qtp", bufs=2))
        ot_p = top.enter_context(tc.tile_pool(name="otp", bufs=2))
        e_p = top.enter_context(tc.tile_pool(name="ep", bufs=14))
        ob_p = top.enter_context(tc.tile_pool(name="obp", bufs=6))
        rec_p = top.enter_context(tc.tile_pool(name="recp", bufs=4))
        osb_p = top.enter_context(tc.tile_pool(name="osbp", bufs=4))

        with ExitStack() as ctx:
            wq_p = ctx.enter_context(tc.tile_pool(name="wqp", bufs=1))
            wqs_t = wq_p.tile([P, DB, NJ], BF16, tag="wqs")   # 16KB/part
            wkv_t = wq_p.tile([P, DB, 2 * P], BF16, tag="wkvs")  # 8KB/part
            xrow = ctx.enter_context(tc.tile_pool(name="xrow", bufs=3))
            xbf = ctx.enter_context(tc.tile_pool(name="xbf", bufs=3))

            # ---- steady state machinery (fills drain between QK batches
            # so the per-engine queues never park on a busy buffer) ----
            fills = deque()       # V-proj + PV chains: drain 1 per QK batch
            fills_big = deque()   # out-proj chunks: 1 per 2 batches

            def drain(n=1, big=False):
                for _ in range(n):
                    if fills:
                        fills.popleft()()
                    elif big and fills_big:
                        fills_big.popleft()()

            def q_proj(qch):
                qt = qt_p.tile([P, G, 4 * P], BF16, tag="QT",
                               name=f"qt{qch}")

                def chain(jb):
                    def go():
                        qp = qpp.tile([P, 4 * P], F32, tag="qp",
                                      name=f"qp{qch}_{jb}")
                        for db in range(DB):
                            nc.tensor.matmul(
                                qp[:], wqs_t[:, db, jb * P:(jb + 1) * P],
                                xbT[:, db, qch * 4 * P:(qch + 1) * 4 * P],
                                start=(db == 0), stop=(db == DB - 1))
                        nc.vector.tensor_copy(qt[:, jb, :], qp[:])
                    return go
                return qt, [chain(jb) for jb in range(G)]

            # ---- head: transpose x, project K and V ----
            # warm-up: after K group g, qch0 QK/exp for sb blocks <= 2g+1
            # (bounded by the E pool: these tiles stay live into segment 0)
            WARM_PLAN = {
                1: [(0, 0), (0, 1), (1, 0), (1, 1)],
                2: [(0, 2), (0, 3), (1, 2), (1, 3),
                    (0, 4), (0, 5), (2, 0), (2, 1)],
            }
            warm = {}
            for g4 in range(NB // 4):
                for k in range(4):
                    nb = g4 * 4 + k
                    xt = xrow.tile([P, D], F32, tag="xrow")
                    nc.sync.dma_start(xt[:], xb[nb * P:(nb + 1) * P, :])
                    if nb == 0:
                        # wkv must be emitted before the first V chain reads
                        # it (program order defines RAW deps); it queues
                        # behind x0 so the transpose pipeline starts first
                        nc.scalar.dma_start(
                            wkv_t[:], wkvs.rearrange("(db p) j -> p db j",
                                                     p=P))
                    elif nb == 2:
                        nc.scalar.dma_start(
                            wqs_t[:], wqs.rearrange("(db p) j -> p db j",
                                                    p=P))
                    xf = xbf.tile([P, D], BF16, tag="xbf")
                    nc.gpsimd.tensor_copy(xf[:], xt[:])
                    for db4 in range(DB // 4):
                        tp = tpp.tile([P, 4, P], BF16, tag="tp")
                        for i in range(4):
                            nc.tensor.transpose(
                                tp[:, i, :],
                                xf[:, (db4 * 4 + i) * P:(db4 * 4 + i + 1) * P],
                                identb[:])
                        nc.vector.tensor_copy(
                            xbT[:, db4 * 4:db4 * 4 + 4, nb * P:(nb + 1) * P],
                            tp[:])
                    vp = qpp.tile([P, 4 * P], F32, tag="qp", name=f"vp{nb}")
                    for db in range(DB):
                        nc.tensor.matmul(
                            vp[:, :P], xbT[:, db, nb * P:(nb + 1) * P],
                            wkv_t[:, db, P:2 * P],
                            start=(db == 0), stop=(db == DB - 1))
                    nc.vector.tensor_copy(Vst[:, nb, 0, :C], vp[:, :C])
                    nc.vector.tensor_copy(Vst[:, nb, 1, :C], vp[:, C:2 * C])
                n0 = g4 * 4 * P
                kp = qpp.tile([P, 4 * P], F32, tag="qp", name=f"kp{g4}")
                for db in range(DB):
                    nc.tensor.matmul(
                        kp[:], wkv_t[:, db, :P], xbT[:, db, n0:n0 + 4 * P],
                        start=(db == 0), stop=(db == DB - 1))
                nc.vector.tensor_copy(KT[:, n0:n0 + 4 * P], kp[:])
                for pair, sbb in WARM_PLAN.get(g4, []):
                    g, dh = pair // 2, pair % 2
                    off = dh * C
                    qk = qk_p.tile([P, 2, 4 * P], F32, tag="qk")
                    for i in range(2):
                        sb = sbb * 2 + i
                        nc.tensor.matmul(
                            qk[:, i, :],
                            KT[off:off + C, sb * P:(sb + 1) * P],
                            qt0[off:off + C, g, :],
                            start=True, stop=True)
                    e = e_p.tile([P, 2, 4 * P], BF16, tag="E")
                    nc.scalar.activation(e[:], qk[:], AF.Exp, scale=SCALE)
                    warm[(pair, sbb)] = e
                if g4 == 0:
                    # Q for the first query chunk, right after its xbT
                    # columns exist, so attention can start at head end
                    qt0, qchains = q_proj(0)
                    for ch in qchains:
                        ch()
            # wps load late: only needed by the first out-proj chunk
            nc.scalar.dma_start(
                wps_t[:], wps.rearrange("(jb p) j -> p jb j", p=P))

            def make_pv(es, pair, dh, obuf, qb, osuf):
                def go():
                    pvt = qpp.tile([P, 4 * P], F32, tag="qp",
                                   name=f"pv{osuf}")
                    pv = pvt[:, :C + 1]
                    for sb in range(NB):
                        nc.tensor.matmul(
                            pv,
                            es[sb // 2][:, sb % 2, qb * P:(qb + 1) * P],
                            Vst[:, sb, dh, :],
                            start=(sb == 0), stop=(sb == NB - 1))
                    rec = rec_p.tile([P, 1], F32, tag="rec")
                    nc.vector.reciprocal(rec[:], pvt[:, C:C + 1])
                    nc.vector.tensor_scalar_mul(
                        obuf[:, pair * C:(pair + 1) * C],
                        pvt[:, :C], rec[:])
                return go

            def make_dchunk(ot, qb, n0, ob, osuf):
                def go():
                    dps = qpp.tile([P, 4 * P], F32, tag="qp",
                                   name=f"d{osuf}_{ob}")
                    for jb in range(G):
                        nc.tensor.matmul(
                            dps[:], ot[:, jb, qb * P:(qb + 1) * P],
                            wps_t[:, jb, ob * 4 * P:(ob + 1) * 4 * P],
                            start=(jb == 0), stop=(jb == G - 1))
                    osb = osb_p.tile([P, 4 * P], F32, tag="osb")
                    nc.vector.tensor_copy(osb[:], dps[:])
                    nc.sync.dma_start(
                        out[n0:n0 + P, ob * 4 * P:(ob + 1) * 4 * P], osb[:])
                return go

            qt = qt0
            for qch in range(4):
                q0 = qch * 4 * P
                qchains = []
                if qch < 3:
                    next_qt, qchains = q_proj(qch + 1)
                obufs = [ob_p.tile([P, NJ], BF16, tag="obuf",
                                   name=f"ob{qch}_{qb}") for qb in range(4)]
                for pair in range(2 * G):
                    if pair < len(qchains):
                        # spread next chunk's Q chains between pairs so the
                        # ACT exp stream never starves behind a 13us block
                        qchains[pair]()
                    g, dh = pair // 2, pair % 2
                    off = dh * C
                    es = []
                    for sbb in range(NB // 2):
                        w = warm.pop((pair, sbb), None) if qch == 0 else None
                        if w is not None:
                            es.append(w)
                            continue
                        qk = qk_p.tile([P, 2, 4 * P], F32, tag="qk")
                        for i in range(2):
                            sb = sbb * 2 + i
                            nc.tensor.matmul(
                                qk[:, i, :],
                                KT[off:off + C, sb * P:(sb + 1) * P],
                                qt[off:off + C, g, :],
                                start=True, stop=True)
                        e = e_p.tile([P, 2, 4 * P], BF16, tag="E")
                        nc.scalar.activation(e[:], qk[:], AF.Exp, scale=SCALE)
                        es.append(e)
                        drain(1, big=(sbb % 2 == 1))
                    for qb in range(4):
                        fills.append(make_pv(es, pair, dh, obufs[qb], qb,
                                             f"{qch}_{pair}_{qb}"))
                while fills or fills_big:
                    drain(big=True)
                ot = ot_p.tile([P, G, 4 * P], BF16, tag="OT",
                               name=f"ot{qch}")
                for qb in range(4):
                    tp = tpp.tile([P, 4, P], BF16, tag="tp")
                    for i in range(4):
                        nc.tensor.transpose(
                            tp[:, i, :], obufs[qb][:, i * P:(i + 1) * P],
                            identb[:])
                    nc.vector.tensor_copy(
                        ot[:, :, qb * P:(qb + 1) * P], tp[:])
                if qch < 3:
                    qt = next_qt
                for qb in range(4):
                    n0 = q0 + qb * P
                    for ob in range(4):
                        f = make_dchunk(ot, qb, n0, ob, f"{qch}_{qb}")
                        if qch < 3:
                            fills_big.append(f)
                        else:
                            f()

    nc.compile()
    return nc


_nc_cache = None


def kernel(x, Wq, Wkv, Wp, bp):
    global _nc_cache
    if _nc_cache is None:
        _nc_cache = build_program()
    nc = _nc_cache
    import ml_dtypes
    x = np.ascontiguousarray(np.asarray(x, dtype=np.float32))
    Wq = np.asarray(Wq, dtype=np.float32)
    Wkv = np.asarray(Wkv, dtype=np.float32)
    Wp = np.asarray(Wp, dtype=np.float32)
    bp = np.asarray(bp, dtype=np.float32)

    in_maps = []
    for c in range(8):
        b, t = c // 4, c % 4
        wq_cols = np.concatenate(
            [Wq[:, (2 * t + dh) * G * C + g * C:(2 * t + dh) * G * C
                + (g + 1) * C] for g in range(G) for dh in range(2)],
            axis=1)
        wkv_cols = np.concatenate(
            [Wkv[:, 2 * t * C:2 * (t + 1) * C],
             Wkv[:, H * C + 2 * t * C:H * C + 2 * (t + 1) * C]], axis=1)
        wp_rows = np.concatenate(
            [Wp[(2 * t + dh) * G * C + g * C:(2 * t + dh) * G * C
                + (g + 1) * C, :] for g in range(G) for dh in range(2)],
            axis=0)
        in_maps.append({
            "xb": x[b],
            "wqs": np.ascontiguousarray(wq_cols).astype(ml_dtypes.bfloat16),
            "wkvs": np.ascontiguousarray(wkv_cols).astype(ml_dtypes.bfloat16),
            "wps": np.ascontiguousarray(wp_rows).astype(ml_dtypes.bfloat16),
        })
    res = run_bass_kernel_spmd(nc, in_maps, list(range(8)))
    outp = np.empty((B, N, D), np.float32)
    for b in range(B):
        acc = res.results[4 * b]["out"].copy()
        for t in range(1, 4):
            acc += res.results[4 * b + t]["out"]
        outp[b] = acc + bp[None, :]
    return outp


# revision 4
# speedup vs baseline: 1.0322x; 1.0322x over previous
"""Trainium2 Bass kernel for GroupedQuerySelfAttention (v3, head-sharded,
software-pipelined).

Sharding: 8 cores = 2 batches x 4 kv-head pairs. Core c: batch b=c//4,
kv-heads {2t, 2t+1} (t=c%4) + their 8 query heads, full sequence. Each
core emits a partial output projection; the host sums the 4 partials
per batch and adds the bias.

Schedule (single PE stream, emission order = per-engine execution
order):
  head:  per 512-row group: load x rows, bf16-convert (gpsimd),
         PE-transpose into xbT, V proj per row-block, K proj per group.
  steady: per 512-query chunk (qch): Q proj (chunk), then 8 (g,dh)
         pairs: QK into 2-bank PSUM batches -> one Exp per batch (ACT
         is the near-critical engine; 2 qk buffers keep it saturated);
         transposed PV ([q,65] output, ones column gives per-partition
         denominators); PV chains + out-proj chunks of the previous
         pair/chunk are interleaved between QK batches so PE never
         parks on a not-yet-free qk buffer.
"""

import numpy as np
from collections import deque
from contextlib import ExitStack

import concourse.bass as bass
import concourse.tile as tile
from concourse import bacc, mybir
from concourse.bass_utils import run_bass_kernel_spmd
from concourse.masks import make_identity

P = 128
B, N, D = 2, 2048, 2048
H, G, C = 8, 4, 64
NB = N // P                   # 16 seq blocks
DB = D // P                   # 16 d blocks
NJ = 2 * G * C                # 512 output cols per core
SCALE = float(1.0 / np.sqrt(H * G))
F32 = mybir.dt.float32
BF16 = mybir.dt.bfloat16
AF = mybir.ActivationFunctionType


def build_program(n_cores=8, phases=None):
    nc = bacc.Bacc("TRN2", target_bir_lowering=False, debug=False,
                   num_devices=n_cores)
    xb = nc.dram_tensor("xb", [N, D], F32, kind="ExternalInput").ap()
    wqs = nc.dram_tensor("wqs", [D, NJ], BF16, kind="ExternalInput").ap()
    wkvs = nc.dram_tensor("wkvs", [D, 2 * P], BF16, kind="ExternalInput").ap()
    wps = nc.dram_tensor("wps", [NJ, D], BF16, kind="ExternalInput").ap()
    out = nc.dram_tensor("out", [N, D], F32, kind="ExternalOutput").ap()

    with tile.TileContext(nc) as tc, ExitStack() as top:
        # ---- persistent stores ----
        store = top.enter_context(tc.tile_pool(name="store", bufs=1))
        KT = store.tile([P, N], BF16, tag="KT")           # 4KB/part
        Vst = store.tile([P, NB, 2, C + 1], BF16, tag="Vst")
        xbT = store.tile([P, DB, N], BF16, tag="xbT")     # 64KB/part
        wps_t = store.tile([P, G, N], BF16, tag="wps")    # 16KB/part
        identb = store.tile([P, P], BF16, tag="identb")
        make_identity(nc, identb[:])
        onesb = store.tile([P, 1], BF16, tag="onesb")
        nc.gpsimd.memset(onesb[:], 1.0)
        nc.vector.tensor_copy(
            Vst[:, :, :, C:C + 1],
            onesb[:, None, None, :].to_broadcast((P, NB, 2, 1)))

        # ---- PSUM pools (8 banks total) ----
        qpp = top.enter_context(           # Q/K/V/D/PV accumulation chains
            tc.tile_pool(name="qpp", bufs=2, space="PSUM"))   # 2 banks
        qk_p = top.enter_context(
            tc.tile_pool(name="qkp", bufs=2, space="PSUM"))   # 4 banks
        tpp = top.enter_context(           # x transposes + OT transposes
            tc.tile_pool(name="tpp", bufs=2, space="PSUM"))   # 2 banks

        # ---- steady-state SBUF pools ----
        qt_p = top.enter_context(tc.tile_pool(name="qtp", bufs=2))
        ot_p = top.enter_context(tc.tile_pool(name="otp", bufs=2))
        e_p = top.enter_context(tc.tile_pool(name="ep", bufs=14))
        ob_p = top.enter_context(tc.tile_pool(name="obp", bufs=6))
        rec_p = top.enter_context(tc.tile_pool(name="recp", bufs=4))
        osb_p = top.enter_context(tc.tile_pool(name="osbp", bufs=4))

        with ExitStack() as ctx:
            wq_p = ctx.enter_context(tc.tile_pool(name="wqp", bufs=1))
            wqs_t = wq_p.tile([P, DB, NJ], BF16, tag="wqs")   # 16KB/part
            wkv_t = wq_p.tile([P, DB, 2 * P], BF16, tag="wkvs")  # 8KB/part
            xrow = ctx.enter_context(tc.tile_pool(name="xrow", bufs=3))
            xbf = ctx.enter_context(tc.tile_pool(name="xbf", bufs=3))

            # ---- steady state machinery (fills drain between QK batches
            # so the per-engine queues never park on a busy buffer) ----
            fills = deque()       # V-proj + PV chains: drain 1 per QK batch
            fills_big = deque()   # out-proj chunks: 1 per 2 batches

            def drain(n=1, big=False):
                for _ in range(n):
                    if fills:
                        fills.popleft()()
                    elif big and fills_big:
                        fills_big.popleft()()

            def q_proj(qch):
                qt = qt_p.tile([P, G, 4 * P], BF16, tag="QT",
                               name=f"qt{qch}")

                def chain(jb):
                    def go():
                        qp = qpp.tile([P, 4 * P], F32, tag="qp",
                                      name=f"qp{qch}_{jb}")
                        for db in range(DB):
                            nc.tensor.matmul(
                                qp[:], wqs_t[:, db, jb * P:(jb + 1) * P],
                                xbT[:, db, qch * 4 * P:(qch + 1) * 4 * P],
                                start=(db == 0), stop=(db == DB - 1))
                        nc.vector.tensor_copy(qt[:, jb, :], qp[:])
                    return go
                return qt, [chain(jb) for jb in range(G)]

            # ---- head: transpose x, project K and V ----
            # warm-up: after K group g, qch0 QK/exp for sb blocks <= 2g+1
            # (bounded by the E pool: these tiles stay live into segment 0)
            WARM_PLAN = {
                1: [(0, 0), (0, 1), (1, 0), (1, 1)],
                2: [(0, 2), (0, 3), (1, 2), (1, 3),
                    (0, 4), (0, 5), (2, 0), (2, 1)],
            }
            warm = {}
            for g4 in range(NB // 4):
                for k in range(4):
                    nb = g4 * 4 + k
                    xt = xrow.tile([P, D], F32, tag="xrow")
                    for q4 in range(4):
                        nc.sync.dma_start(
                            xt[:, q4 * D // 4:(q4 + 1) * D // 4],
                            xb[nb * P:(nb + 1) * P,
                               q4 * D // 4:(q4 + 1) * D // 4])
                    if nb == 0:
                        # wkv must be emitted before the first V chain reads
                        # it (program order defines RAW deps); it queues
                        # behind x0 so the transpose pipeline starts first
                        nc.scalar.dma_start(
                            wkv_t[:], wkvs.rearrange("(db p) j -> p db j",
                                                     p=P))
                    elif nb == 2:
                        nc.scalar.dma_start(
                            wqs_t[:], wqs.rearrange("(db p) j -> p db j",
                                                    p=P))
                    xf = xbf.tile([P, D], BF16, tag="xbf")
                    for q4 in range(4):
                        lo, hi = q4 * D // 4, (q4 + 1) * D // 4
                        if q4 % 2 == 0:
                            nc.scalar.copy(xf[:, lo:hi], xt[:, lo:hi])
                        else:
                            nc.gpsimd.tensor_copy(xf[:, lo:hi], xt[:, lo:hi])
                    for db4 in range(DB // 4):
                        tp = tpp.tile([P, 4, P], BF16, tag="tp")
                        for i in range(4):
                            nc.tensor.transpose(
                                tp[:, i, :],
                                xf[:, (db4 * 4 + i) * P:(db4 * 4 + i + 1) * P],
                                identb[:])
                        nc.vector.tensor_copy(
                            xbT[:, db4 * 4:db4 * 4 + 4, nb * P:(nb + 1) * P],
                            tp[:])
                    vp = qpp.tile([P, 4 * P], F32, tag="qp", name=f"vp{nb}")
                    for db in range(DB):
                        nc.tensor.matmul(
                            vp[:, :P], xbT[:, db, nb * P:(nb + 1) * P],
                            wkv_t[:, db, P:2 * P],
                            start=(db == 0), stop=(db == DB - 1))
                    nc.vector.tensor_copy(Vst[:, nb, 0, :C], vp[:, :C])
                    nc.vector.tensor_copy(Vst[:, nb, 1, :C], vp[:, C:2 * C])
                n0 = g4 * 4 * P
                kp = qpp.tile([P, 4 * P], F32, tag="qp", name=f"kp{g4}")
                for db in range(DB):
                    nc.tensor.matmul(
                        kp[:], wkv_t[:, db, :P], xbT[:, db, n0:n0 + 4 * P],
                        start=(db == 0), stop=(db == DB - 1))
                nc.vector.tensor_copy(KT[:, n0:n0 + 4 * P], kp[:])
                for pair, sbb in WARM_PLAN.get(g4, []):
                    g, dh = pair // 2, pair % 2
                    off = dh * C
                    qk = qk_p.tile([P, 2, 4 * P], F32, tag="qk")
                    for i in range(2):
                        sb = sbb * 2 + i
                        nc.tensor.matmul(
                            qk[:, i, :],
                            KT[off:off + C, sb * P:(sb + 1) * P],
                            qt0[off:off + C, g, :],
                            start=True, stop=True)
                    e = e_p.tile([P, 2, 4 * P], BF16, tag="E")
                    nc.scalar.activation(e[:], qk[:], AF.Exp, scale=SCALE)
                    warm[(pair, sbb)] = e
                if g4 == 0:
                    # Q for the first query chunk, right after its xbT
                    # columns exist, so attention can start at head end
                    qt0, qchains = q_proj(0)
                    for ch in qchains:
                        ch()
            # wps load late: only needed by the first out-proj chunk
            nc.scalar.dma_start(
                wps_t[:], wps.rearrange("(jb p) j -> p jb j", p=P))

            def make_pv(es, pair, dh, obuf, qb, osuf):
                def go():
                    pvt = qpp.tile([P, 4 * P], F32, tag="qp",
                                   name=f"pv{osuf}")
                    pv = pvt[:, :C + 1]
                    for sb in range(NB):
                        nc.tensor.matmul(
                            pv,
                            es[sb // 2][:, sb % 2, qb * P:(qb + 1) * P],
                            Vst[:, sb, dh, :],
                            start=(sb == 0), stop=(sb == NB - 1))
                    rec = rec_p.tile([P, 1], F32, tag="rec")
                    nc.vector.reciprocal(rec[:], pvt[:, C:C + 1])
                    nc.vector.tensor_scalar_mul(
                        obuf[:, pair * C:(pair + 1) * C],
                        pvt[:, :C], rec[:])
                return go

            def make_dchunk(ot, qb, n0, ob, osuf):
                def go():
                    dps = qpp.tile([P, 4 * P], F32, tag="qp",
                                   name=f"d{osuf}_{ob}")
                    for jb in range(G):
                        nc.tensor.matmul(
                            dps[:], ot[:, jb, qb * P:(qb + 1) * P],
                            wps_t[:, jb, ob * 4 * P:(ob + 1) * 4 * P],
                            start=(jb == 0), stop=(jb == G - 1))
                    osb = osb_p.tile([P, 4 * P], F32, tag="osb")
                    nc.vector.tensor_copy(osb[:], dps[:])
                    nc.sync.dma_start(
                        out[n0:n0 + P, ob * 4 * P:(ob + 1) * 4 * P], osb[:])
                return go

            qt = qt0
            for qch in range(4):
                q0 = qch * 4 * P
                qchains = []
                if qch < 3:
                    next_qt, qchains = q_proj(qch + 1)
                obufs = [ob_p.tile([P, NJ], BF16, tag="obuf",
                                   name=f"ob{qch}_{qb}") for qb in range(4)]
                for pair in range(2 * G):
                    if pair < len(qchains):
                        # spread next chunk's Q chains between pairs so the
                        # ACT exp stream never starves behind a 13us block
                        qchains[pair]()
                    g, dh = pair // 2, pair % 2
                    off = dh * C
                    es = []
                    for sbb in range(NB // 2):
                        w = warm.pop((pair, sbb), None) if qch == 0 else None
                        if w is not None:
                            es.append(w)
                            continue
                        qk = qk_p.tile([P, 2, 4 * P], F32, tag="qk")
                        for i in range(2):
                            sb = sbb * 2 + i
                            nc.tensor.matmul(
                                qk[:, i, :],
                                KT[off:off + C, sb * P:(sb + 1) * P],
                                qt[off:off + C, g, :],
                                start=True, stop=True)
                        e = e_p.tile([P, 2, 4 * P], BF16, tag="E")
                        nc.scalar.activation(e[:], qk[:], AF.Exp, scale=SCALE)
                        es.append(e)
                        drain(1, big=(sbb % 2 == 1))
                    for qb in range(4):
                        fills.append(make_pv(es, pair, dh, obufs[qb], qb,
                                             f"{qch}_{pair}_{qb}"))
                while fills:
                    drain()
                ot = ot_p.tile([P, G, 4 * P], BF16, tag="OT",
                               name=f"ot{qch}")

                def make_otr(ot, obuf, qb):
                    def go():
                        tp = tpp.tile([P, 4, P], BF16, tag="tp")
                        for i in range(4):
                            nc.tensor.transpose(
                                tp[:, i, :], obuf[:, i * P:(i + 1) * P],
                                identb[:])
                        nc.vector.tensor_copy(
                            ot[:, :, qb * P:(qb + 1) * P], tp[:])
                    return go
                for qb in range(4):
                    if qch < 3:
                        fills_big.append(make_otr(ot, obufs[qb], qb))
                    else:
                        make_otr(ot, obufs[qb], qb)()
                if qch < 3:
                    qt = next_qt
                for qb in range(4):
                    n0 = q0 + qb * P
                    for ob in range(4):
                        f = make_dchunk(ot, qb, n0, ob, f"{qch}_{qb}")
                        if qch < 3:
                            fills_big.append(f)
                        else:
                            f()
                if qch == 3:
                    while fills_big:
                        drain(big=True)

    nc.compile()
    return nc


_nc_cache = None


def kernel(x, Wq, Wkv, Wp, bp):
    global _nc_cache
    if _nc_cache is None:
        _nc_cache = build_program()
    nc = _nc_cache
    import ml_dtypes
    x = np.ascontiguousarray(np.asarray(x, dtype=np.float32))
    Wq = np.asarray(Wq, dtype=np.float32)
    Wkv = np.asarray(Wkv, dtype=np.float32)
    Wp = np.asarray(Wp, dtype=np.float32)
    bp = np.asarray(bp, dtype=np.float32)

    in_maps = []
    for c in range(8):
        b, t = c // 4, c % 4
        wq_cols = np.concatenate(
            [Wq[:, (2 * t + dh) * G * C + g * C:(2 * t + dh) * G * C
                + (g + 1) * C] for g in range(G) for dh in range(2)],
            axis=1)
        wkv_cols = np.concatenate(
            [Wkv[:, 2 * t * C:2 * (t + 1) * C],
             Wkv[:, H * C + 2 * t * C:H * C + 2 * (t + 1) * C]], axis=1)
        wp_rows = np.concatenate(
            [Wp[(2 * t + dh) * G * C + g * C:(2 * t + dh) * G * C
                + (g + 1) * C, :] for g in range(G) for dh in range(2)],
            axis=0)
        in_maps.append({
            "xb": x[b],
            "wqs": np.ascontiguousarray(wq_cols).astype(ml_dtypes.bfloat16),
            "wkvs": np.ascontiguousarray(wkv_cols).astype(ml_dtypes.bfloat16),
            "wps": np.ascontiguousarray(wp_rows).astype(ml_dtypes.bfloat16),
        })
    res = run_bass_kernel_spmd(nc, in_maps, list(range(8)))
    outp = np.empty((B, N, D), np.float32)
    for b in range(B):
        acc = res.results[4 * b]["out"].copy()
        for t in range(1, 4):
            acc += res.results[4 * b + t]["out"]
        outp[b] = acc + bp[None, :]
    return outp
